# revision 1
# baseline (speedup 1.0000x reference)
"""TBCNN tree-convolution layer on 8 trn2 NeuronCores (data-parallel).

Math (validated against reference to 1.6e-7):
  res[b,n] = X[b,n]@w_t + P[b,n]@w_l + Q[b,n]@(w_r-w_l) + conv -> leaky_relu(0.01)
  P = S_P @ X, Q = S_Q @ X  with S_* (512x512) adjacency built from children:
  S_P[n,m] = sum_j has[n,j]*[c[n,j]=m];  S_Q[n,m] = sum_j w1[n,j]*[c[n,j]=m]
  w1 = has*(a*j + b*[j==0]); a = 1/(ns-1) if ns>1 else 0; b = 0.5*[ns==1]

Sharding: batch (tree) axis split 4 trees/core across 8 cores via pmap;
weights replicated. The gather is reformulated as dense adjacency matmuls
(each node referenced ~16x -> PE-friendly, no data-dependent addressing).
"""

import numpy as np

B, N, C, D, O = 32, 512, 16, 256, 256
NCORES = 8
TPC = B // NCORES

_compiled = None


def _host_prep(nodes, w_t, w_l, w_r, conv, children):
    nodes = np.asarray(nodes, np.float32)
    ch = np.asarray(children).astype(np.int64)
    has = ch > 0
    ns = has.sum(-1)
    a = np.where(ns > 1, 1.0 / np.maximum(ns - 1, 1), 0.0)
    bco = np.where(ns == 1, 0.5, 0.0)
    jar = np.arange(C, dtype=np.float64)
    w0 = has.astype(np.float64)
    w1 = has * (a[..., None] * jar + bco[..., None] * (jar == 0))

    bi, ni, ji = np.nonzero(has)
    mi = ch[bi, ni, ji]
    sp = np.zeros((B, N, N), np.float32)
    sq = np.zeros((B, N, N), np.float32)
    np.add.at(sp, (bi, ni, mi), w0[bi, ni, ji])
    np.add.at(sq, (bi, ni, mi), w1[bi, ni, ji])
    return nodes, sp, sq


def kernel(**inputs):
    global _compiled
    import jax
    import jax.numpy as jnp

    nodes, sp, sq = _host_prep(**inputs)
    w_t = np.asarray(inputs["w_t"], np.float32)
    w_l = np.asarray(inputs["w_l"], np.float32)
    w_rl = np.asarray(inputs["w_r"], np.float32) - w_l
    conv = np.asarray(inputs["conv"], np.float32)

    if _compiled is None:
        def per_core(x, s_p, s_q, wt, wl, wrl, cv):
            # x: (TPC,N,D)  s_*: (TPC,N,N)
            p = jnp.einsum("tnm,tmd->tnd", s_p, x)
            q = jnp.einsum("tnm,tmd->tnd", s_q, x)
            res = x @ wt + p @ wl + q @ wrl + cv
            return jnp.where(res > 0, res, 0.01 * res)

        _compiled = jax.pmap(
            per_core,
            in_axes=(0, 0, 0, None, None, None, None),
            devices=jax.devices()[:NCORES],
        )

    xs = nodes.reshape(NCORES, TPC, N, D)
    sps = sp.reshape(NCORES, TPC, N, N)
    sqs = sq.reshape(NCORES, TPC, N, N)
    out = _compiled(xs, sps, sqs, w_t, w_l, w_rl, conv)
    return np.asarray(out).reshape(B, N, O)



# revision 7
# speedup vs baseline: 3.2277x; 3.2277x over previous
"""TBCNN tree-convolution layer on 8 trn2 NeuronCores — Bass kernel.

Math (validated against reference):
  out[b,n] = leaky_relu(X[b,n]@w_t + P[b,n]@w_l + Q[b,n]@(w_r-w_l) + conv, 0.01)
  P[b,n] = sum_j w0[b,n,j] * nodes[b, c[b,n,j]]   (w0 = has_child)
  Q[b,n] = sum_j w1[b,n,j] * nodes[b, c[b,n,j]]   (w1 = eta_r coefficient)

The wire (axon tunnel, ~80MB/s, ~70-90ms per round trip) dominates, so the
kernel sends only bf16 nodes + int32 children + bf16 coefficients (~12MB up,
8MB bf16 down) and does the gather on-device via indirect DMA.

Sharding: batch (tree) axis, 4 trees/core across 8 cores. Weights replicated.
"""

import numpy as np

B, N, C, D, O = 32, 512, 16, 256, 256
NCORES = 8
TPC = B // NCORES          # trees per core
ROWS = TPC * N             # per-core node rows (2048)
P = 128                    # SBUF partitions

_STATE = None


# ---------------------------------------------------------------- host utils

def _f32_to_bf16_bits(a: np.ndarray) -> np.ndarray:
    """f32 -> bf16 (round to nearest even), returned as uint16 bits."""
    u = np.ascontiguousarray(a, np.float32).view(np.uint32)
    rounded = u + 0x7FFF + ((u >> 16) & 1)
    return (rounded >> 16).astype(np.uint16)


def _to_bf16(a: np.ndarray):
    import ml_dtypes
    return _f32_to_bf16_bits(a).view(ml_dtypes.bfloat16)


def _bf16_to_f32(a: np.ndarray) -> np.ndarray:
    u = np.asarray(a).view(np.uint16).astype(np.uint32) << 16
    return u.view(np.float32)


def _coefs(children: np.ndarray):
    """w0 (eta_t-complement mask) and w1 (eta_r) per (b, n, j)."""
    has = children > 0
    ns = has.sum(-1)
    a = np.where(ns > 1, 1.0 / np.maximum(ns - 1, 1), 0.0)
    bco = np.where(ns == 1, 0.5, 0.0)
    jar = np.arange(C, dtype=np.float32)
    w0 = has.astype(np.float32)
    w1 = (has * (a[..., None] * jar + bco[..., None] * (jar == 0))).astype(np.float32)
    return w0, w1


# ---------------------------------------------------------------- bass kernel

def _build_bass():
    from contextlib import ExitStack
    import concourse.bacc as bacc
    import concourse.bass as bass
    import concourse.tile as tile
    from concourse import mybir
    from concourse.masks import make_identity

    dt = mybir.dt
    nc = bacc.Bacc(
        "TRN2",
        target_bir_lowering=False,
        debug=False,
        num_devices=NCORES,
    )

    nodes_d = nc.dram_tensor("nodes", [ROWS, D], dt.bfloat16, kind="ExternalInput")
    ch_d = nc.dram_tensor("children", [ROWS, C], dt.int32, kind="ExternalInput")
    w01_d = nc.dram_tensor("w01", [ROWS, 2 * C], dt.bfloat16, kind="ExternalInput")
    wts_d = nc.dram_tensor("wts", [D, 3 * O], dt.bfloat16, kind="ExternalInput")
    conv_d = nc.dram_tensor("convb", [1, O], dt.bfloat16, kind="ExternalInput")
    out_d = nc.dram_tensor("out", [ROWS, O], dt.bfloat16, kind="ExternalOutput")

    NCHUNK = ROWS // P           # 16 chunks of 128 nodes
    CPT = N // P                 # chunks per tree (4)

    with tile.TileContext(nc) as tc, ExitStack() as ctx:
        wpool = ctx.enter_context(tc.tile_pool(name="w", bufs=1))
        wts_sb = wpool.tile([P, 2 * 3 * O], dt.bfloat16)
        # [:, :768] = weight rows 0..127, [:, 768:] = rows 128..255 of [w_t|w_l|w_rl]
        nc.sync.dma_start(wts_sb[:, 0:768], wts_d[0:P, :])
        nc.sync.dma_start(wts_sb[:, 768:1536], wts_d[P : 2 * P, :])
        conv_sb = wpool.tile([1, O], dt.bfloat16)
        nc.sync.dma_start(conv_sb[:], conv_d[:])
        ones_sb = wpool.tile([1, P], dt.bfloat16)
        nc.vector.memset(ones_sb[:], 1.0)
        ident_bf = wpool.tile([P, P], dt.bfloat16)
        make_identity(nc, ident_bf[:])
        ident_f32 = wpool.tile([P, P], dt.float32)
        make_identity(nc, ident_f32[:])

        lpool = ctx.enter_context(tc.tile_pool(name="loads", bufs=3))
        epool = ctx.enter_context(tc.tile_pool(name="emb", bufs=4))
        apool = ctx.enter_context(tc.tile_pool(name="acc", bufs=2))
        tpool = ctx.enter_context(tc.tile_pool(name="trans", bufs=2))
        opool = ctx.enter_context(tc.tile_pool(name="outs", bufs=3))
        pspool = ctx.enter_context(tc.tile_pool(name="psum", bufs=2, space="PSUM"))
        pstp = ctx.enter_context(tc.tile_pool(name="psumt", bufs=1, space="PSUM"))

        for chunk in range(NCHUNK):
            r0 = chunk * P
            tree = chunk // CPT

            x_sb = lpool.tile([P, D], dt.bfloat16, tag="x")
            nc.sync.dma_start(x_sb[:], nodes_d[r0 : r0 + P, :])
            ch_sb = lpool.tile([P, C], dt.int32, tag="ch")
            nc.sync.dma_start(ch_sb[:], ch_d[r0 : r0 + P, :])
            w01_bf = lpool.tile([P, 2 * C], dt.bfloat16, tag="w01bf")
            nc.sync.dma_start(w01_bf[:], w01_d[r0 : r0 + P, :])
            w01_sb = lpool.tile([P, 2 * C], dt.float32, tag="w01")
            nc.vector.tensor_copy(w01_sb[:], w01_bf[:])

            pacc = apool.tile([P, D], dt.float32, tag="pacc")
            qacc = apool.tile([P, D], dt.float32, tag="qacc")
            for j in range(C):
                emb = epool.tile([P, D], dt.bfloat16, tag="emb")
                nc.gpsimd.indirect_dma_start(
                    out=emb[:],
                    out_offset=None,
                    in_=nodes_d[:, :],
                    in_offset=bass.IndirectOffsetOnAxis(ap=ch_sb[:, j : j + 1], axis=0),
                    element_offset=tree * N * D,
                )
                if j == 0:
                    nc.vector.tensor_scalar(
                        out=pacc[:], in0=emb[:], scalar1=w01_sb[:, 0:1],
                        scalar2=None, op0=mybir.AluOpType.mult,
                    )
                    nc.vector.tensor_scalar(
                        out=qacc[:], in0=emb[:], scalar1=w01_sb[:, C : C + 1],
                        scalar2=None, op0=mybir.AluOpType.mult,
                    )
                else:
                    tmp_p = epool.tile([P, D], dt.float32, tag="tmp_p")
                    nc.scalar.activation(
                        out=tmp_p[:], in_=emb[:],
                        func=mybir.ActivationFunctionType.Copy,
                        scale=w01_sb[:, j : j + 1],
                    )
                    nc.vector.tensor_add(pacc[:], pacc[:], tmp_p[:])
                    tmp_q = epool.tile([P, D], dt.float32, tag="tmp_q")
                    nc.vector.tensor_scalar(
                        out=tmp_q[:], in0=emb[:], scalar1=w01_sb[:, C + j : C + j + 1],
                        scalar2=None, op0=mybir.AluOpType.mult,
                    )
                    nc.vector.tensor_add(qacc[:], qacc[:], tmp_q[:])

            # transpose X, P, Q into (d, n) layout for the output matmuls
            xt = tpool.tile([P, D], dt.bfloat16, tag="xt")
            pt = tpool.tile([P, D], dt.bfloat16, tag="pt")
            qt = tpool.tile([P, D], dt.bfloat16, tag="qt")
            for dc in range(2):
                sl = slice(dc * P, (dc + 1) * P)
                tp_x = pstp.tile([P, P], dt.bfloat16, tag="tp_x")
                nc.tensor.transpose(out=tp_x[:], in_=x_sb[:, sl], identity=ident_bf[:])
                nc.scalar.copy(xt[:, sl], tp_x[:])
                tp_p = pstp.tile([P, P], dt.float32, tag="tp_p")
                nc.tensor.transpose(out=tp_p[:], in_=pacc[:, sl], identity=ident_f32[:])
                nc.scalar.copy(pt[:, sl], tp_p[:])
                tp_q = pstp.tile([P, P], dt.float32, tag="tp_q")
                nc.tensor.transpose(out=tp_q[:], in_=qacc[:, sl], identity=ident_f32[:])
                nc.scalar.copy(qt[:, sl], tp_q[:])

            # out[n, o] = Xt.T@w_t + Pt.T@w_l + Qt.T@w_rl + ones.T@conv
            out_ps = pspool.tile([P, O], dt.float32, tag="ops")
            nc.tensor.matmul(out=out_ps[:], lhsT=xt[:, 0:P], rhs=wts_sb[:, 0:256],
                             start=True, stop=False)
            nc.tensor.matmul(out=out_ps[:], lhsT=xt[:, P:D], rhs=wts_sb[:, 768:1024],
                             start=False, stop=False)
            nc.tensor.matmul(out=out_ps[:], lhsT=pt[:, 0:P], rhs=wts_sb[:, 256:512],
                             start=False, stop=False)
            nc.tensor.matmul(out=out_ps[:], lhsT=pt[:, P:D], rhs=wts_sb[:, 1024:1280],
                             start=False, stop=False)
            nc.tensor.matmul(out=out_ps[:], lhsT=qt[:, 0:P], rhs=wts_sb[:, 512:768],
                             start=False, stop=False)
            nc.tensor.matmul(out=out_ps[:], lhsT=qt[:, P:D], rhs=wts_sb[:, 1280:1536],
                             start=False, stop=False)
            nc.tensor.matmul(out=out_ps[:], lhsT=ones_sb[:], rhs=conv_sb[:],
                             start=False, stop=True)

            out_sb = opool.tile([P, O], dt.bfloat16, tag="osb")
            small = opool.tile([P, O], dt.float32, tag="small")
            nc.scalar.mul(small[:], out_ps[:], 0.01)
            nc.vector.tensor_tensor(
                out=out_sb[:], in0=out_ps[:], in1=small[:],
                op=mybir.AluOpType.max,
            )
            nc.sync.dma_start(out_d[r0 : r0 + P, :], out_sb[:])

    nc.compile()
    if not nc.is_finalized():
        nc.finalize()
    return nc


# ---------------------------------------------------------------- jax glue

def _build_exec():
    import jax
    from jax.sharding import Mesh, PartitionSpec
    from jax.experimental.shard_map import shard_map
    from concourse import bass2jax, mybir

    nc = _build_bass()
    bass2jax.install_neuronx_cc_hook()

    in_names, out_names, out_avals = [], [], []
    partition_name = (
        nc.partition_id_tensor.name if nc.partition_id_tensor is not None else None
    )
    for alloc in nc.m.functions[0].allocations:
        if not isinstance(alloc, mybir.MemoryLocationSet):
            continue
        name = alloc.memorylocations[0].name
        if alloc.kind == "ExternalInput":
            if name != partition_name:
                in_names.append(name)
        elif alloc.kind == "ExternalOutput":
            out_names.append(name)
            out_avals.append(
                jax.core.ShapedArray(
                    tuple(alloc.tensor_shape), mybir.dt.np(alloc.dtype)
                )
            )
    if partition_name is not None:
        in_names.append(partition_name)

    devices = jax.devices()[:NCORES]
    mesh = Mesh(np.asarray(devices), ("core",))
    SHARDED = {"nodes", "children", "w01", "out"}

    def _body(*args):
        operands = list(args)
        if partition_name is not None:
            operands.append(bass2jax.partition_id_tensor())
        outs = bass2jax._bass_exec_p.bind(
            *operands,
            out_avals=tuple(out_avals),
            in_names=tuple(in_names),
            out_names=tuple(out_names),
            lowering_input_output_aliases=(),
            sim_require_finite=True,
            sim_require_nnan=True,
            nc=nc,
        )
        return tuple(outs)

    n_real = len(in_names) - (1 if partition_name else 0)
    in_specs = tuple(
        PartitionSpec("core") if in_names[i] in SHARDED else PartitionSpec()
        for i in range(n_real)
    )
    out_specs = tuple(
        PartitionSpec("core") if name in SHARDED else PartitionSpec()
        for name in out_names
    )
    fn = jax.jit(
        shard_map(_body, mesh=mesh, in_specs=in_specs, out_specs=out_specs,
                  check_rep=False)
    )
    return {"fn": fn, "in_names": in_names[:n_real], "out_names": out_names}


# ---------------------------------------------------------------- entry point

def kernel(**inputs):
    global _STATE

    nodes = np.ascontiguousarray(np.asarray(inputs["nodes"], np.float32))
    children = np.asarray(inputs["children"]).astype(np.int32)
    w_t = np.asarray(inputs["w_t"], np.float32)
    w_l = np.asarray(inputs["w_l"], np.float32)
    w_r = np.asarray(inputs["w_r"], np.float32)
    conv = np.asarray(inputs["conv"], np.float32)

    w0, w1 = _coefs(children)
    w01 = _to_bf16(np.concatenate([w0, w1], axis=-1)).reshape(B * N, 2 * C)
    nodes_bf = _to_bf16(nodes).reshape(B * N, D)
    ch32 = np.ascontiguousarray(children.reshape(B * N, C))
    wts = _to_bf16(np.concatenate([w_t, w_l, w_r - w_l], axis=1))
    conv_bf = _to_bf16(conv).reshape(1, O)

    if _STATE is None:
        _STATE = _build_exec()

    arrays = {
        "nodes": nodes_bf,
        "children": ch32,
        "w01": w01,
        "wts": wts,
        "convb": conv_bf,
    }
    args = [arrays[name] for name in _STATE["in_names"]]
    out = _STATE["fn"](*args)[0]
    out_np = np.asarray(out)
    return _bf16_to_f32(out_np).reshape(B, N, O)
